# revision 18
# baseline (speedup 1.0000x reference)
"""Trainium2 Bass kernel for nn_CumulativeProbingDense.

Computation (see reference):
    h      = sum_l softmax(mixing_weights)[l] * x[:, l] * gamma   # [B, S, F]
    h1     = relu(h @ W1.T + b1)                                  # [B, S, H]
    h2     = relu(h1 @ W2.T + b2)                                 # [B, S, H]
    pooled = (h2 * mask).sum(S) / lengths                         # [B, H]
    logits = pooled @ Wl.T + bl                                   # [B, NL]

Sharding: tokens masked out by `lengths` (t >= lengths[b]) cannot affect the
output, so only the sum(lengths) valid tokens are processed.  The host packs
the valid (sample, token) pairs of ALL samples into one stream, split evenly
across the 8 cores (token-balanced data parallel).  Each core streams its
packed x slice ([t_tiles*128 tokens, L, F], token-major so DMA runs are
contiguous), computes the layer mix + MLP, and mask-matmul-pools per sample.
Host combines the per-core pooled partials and applies the tiny final linear.

Device pipeline per 128-token tile:
  - layer mix on TensorE: PSUM-accumulated matmuls with scaled-identity
    stationary; optionally a few trailing layers accumulate on the DVE
  - PE-transpose mixed tile into hT [feat part, token free]
  - mm1 (W1T stationary) -> relu+b1 on ScalarE -> h1 [hid part, token free]
  - mm2 with h1 chunks as STATIONARY and W2T moving -> h2 [token part, hid
    free]; b2 enters the same PSUM group as a ones x (b2/128) matmul
  - pooling: matmul with the per-sample 0/1 mask [token, 16] stationary,
    PSUM-accumulated over all tiles -> pooled [16, 256] per core
"""

import numpy as np

import concourse.bass as bass
import concourse.tile as tile
from concourse import mybir
from concourse.bass_utils import run_bass_kernel_spmd
from contextlib import ExitStack

F32 = mybir.dt.float32
F32R = mybir.dt.float32r

N_CORES = 8
B, L, S, F = 16, 13, 1024, 768
H, NL = 256, 7
P = 128                       # SBUF partitions
FC = F // P                   # feature chunks of 128
HC = H // P                   # hidden chunks of 128
CWMAX = 2 * P                 # token width of one MLP chunk

# matmul input dtype: float32r streams at 1 cycle/row (moving dim >= 256)
# vs plain float32's 4 cycles/row. fp32r rounds the operands (TF32-like).
MM_DT = F32R


def _split_excess_waits(nc, max_waits=1):
    """walrus (CoreV3) rejects instructions carrying more than a couple of
    sync waits (e.g. the TileContext exit drain). Hoist excess waits onto
    standalone NoOps inserted before the offending instruction."""
    n_fixed = 0
    for f in nc.m.functions:
        for bb in f.blocks:
            out, changed = [], False
            for inst in bb.instructions:
                si = getattr(inst, "sync_info", None)
                if si is not None and len(si.on_wait) > max_waits:
                    waits = list(si.on_wait)
                    for j, w in enumerate(waits[max_waits:]):
                        out.append(mybir.InstNoOp(
                            name=f"{inst.name}-wsplit{j}",
                            engine=inst.engine, ins=[], outs=[],
                            sync_info=mybir.SyncInfo(on_wait=[w], on_update=[]),
                        ))
                    inst.sync_info = mybir.SyncInfo(
                        on_wait=waits[:max_waits], on_update=list(si.on_update))
                    changed = True
                    n_fixed += 1
                out.append(inst)
            if changed:
                bb.instructions = out
    return n_fixed


def _plan_packing(lengths):
    """Token-balanced packing of all valid (sample, token) pairs onto cores.

    Returns (t_tiles, last_tw, b_idx [n_cores, cap], t_idx [n_cores, cap],
    valid [n_cores, cap]) with cap = (t_tiles-1)*128 + last_tw slots per
    core (the final token tile is partial); pad slots point at (0, 0) with
    valid=0."""
    lengths = np.asarray(lengths).astype(np.int64)
    total = int(lengths.sum())
    cap = max(1, -(-total // N_CORES))
    t_tiles = -(-cap // P)
    last_tw = cap - (t_tiles - 1) * P
    bs = np.repeat(np.arange(B, dtype=np.int64), lengths)
    ts = np.concatenate([np.arange(n, dtype=np.int64) for n in lengths])
    pad = N_CORES * cap - total
    bs = np.concatenate([bs, np.zeros(pad, np.int64)])
    ts = np.concatenate([ts, np.zeros(pad, np.int64)])
    val = np.concatenate([np.ones(total, np.float32), np.zeros(pad, np.float32)])
    return (t_tiles, last_tw, bs.reshape(N_CORES, cap),
            ts.reshape(N_CORES, cap), val.reshape(N_CORES, cap))


def build_program(n_layers: int, t_tiles: int, last_tw: int = P,
                  split_waits: bool = True,
                  hw_loop_repeat: int | None = None,
                  mix_dve_layers: int = 0,
                  x_bufs: int = 3,
                  dma_pieces=2,
                  dma_engines: int = 1) -> bass.Bass:
    # mix_dve_layers: trailing layers accumulated on the DVE (axpy) instead
    # of the TensorE, to balance PE vs DVE occupancy.
    n_pe_layers = n_layers - min(mix_dve_layers, n_layers - 1)
    cap = (t_tiles - 1) * P + last_tw
    nc = bass.Bass("TRN2", target_bir_lowering=False, debug=False, num_devices=1)

    xp_d = nc.dram_tensor("xp", [cap, n_layers * F], F32R, kind="ExternalInput").ap()
    seye_d = nc.dram_tensor("seye", [P, n_layers * P], F32R, kind="ExternalInput").ap()
    svec_d = nc.dram_tensor("svec", [P, n_layers], F32, kind="ExternalInput").ap()
    ident_d = nc.dram_tensor("ident", [P, P], F32, kind="ExternalInput").ap()
    w1t_d = nc.dram_tensor("w1t", [P, FC * H], F32R, kind="ExternalInput").ap()
    w2t_d = nc.dram_tensor("w2t", [P, HC * H], F32R, kind="ExternalInput").ap()
    b1_d = nc.dram_tensor("b1", [P, HC], F32, kind="ExternalInput").ap()
    b2rep_d = nc.dram_tensor("b2rep", [P, H], F32R, kind="ExternalInput").ap()
    ones_d = nc.dram_tensor("ones", [P, P], F32R, kind="ExternalInput").ap()
    msk_d = nc.dram_tensor("msk", [P, t_tiles * B], F32R, kind="ExternalInput").ap()
    out_d = nc.dram_tensor("out", [B, H], F32, kind="ExternalOutput").ap()

    with TileKernel(nc) as (tc, ctx):
        const = ctx.enter_context(tc.tile_pool(name="const", bufs=1))
        xpool = ctx.enter_context(tc.tile_pool(name="x", bufs=x_bufs))
        hpool = ctx.enter_context(tc.tile_pool(name="h", bufs=3))
        htpool = ctx.enter_context(tc.tile_pool(name="ht", bufs=2))
        h1pool = ctx.enter_context(tc.tile_pool(name="h1", bufs=2))
        h2pool = ctx.enter_context(tc.tile_pool(name="h2", bufs=2))
        pmix0 = ctx.enter_context(tc.tile_pool(name="pmix0", bufs=2, space="PSUM"))
        pmix1 = ctx.enter_context(tc.tile_pool(name="pmix1", bufs=2, space="PSUM"))
        pshared = ctx.enter_context(tc.tile_pool(name="pshared", bufs=3, space="PSUM"))
        ptr = pmm1 = pmm2 = pshared
        ppool = ctx.enter_context(tc.tile_pool(name="ppool", bufs=1, space="PSUM"))

        # ---- constants into SBUF via SWDGE (gpsimd), keeping the HWDGE
        # rings free for the x stream ----
        seye = const.tile([P, n_layers * P], F32R)
        nc.gpsimd.dma_start(seye[:], seye_d[:])
        svec = const.tile([P, n_layers], F32)
        nc.gpsimd.dma_start(svec[:], svec_d[:])
        ident = const.tile([P, P], F32)
        nc.gpsimd.dma_start(ident[:], ident_d[:])
        w1t = const.tile([P, FC * H], F32R)
        nc.gpsimd.dma_start(w1t[:], w1t_d[:])
        w2t = const.tile([P, HC * H], F32R)
        nc.gpsimd.dma_start(w2t[:], w2t_d[:])
        b1 = const.tile([P, HC], F32)
        nc.gpsimd.dma_start(b1[:], b1_d[:])
        b2rep = const.tile([P, H], F32R)
        nc.gpsimd.dma_start(b2rep[:], b2rep_d[:])
        ones = const.tile([P, P], F32R)
        nc.gpsimd.dma_start(ones[:], ones_d[:])
        msk = const.tile([P, t_tiles * B], F32R)
        nc.gpsimd.dma_start(msk[:], msk_d[:])

        pooled_sb = const.tile([B, H], F32)

        # MLP chunks of up to 2 token tiles (moving dim 256); the final
        # tile gets its own chunk so the post-DMA tail chain stays short
        chunk_plan = []
        rem = t_tiles - 1
        t = 0
        while t + 1 < rem:
            chunk_plan.append((t, 2))
            t += 2
        if t < rem:
            chunk_plan.append((t, 1))
            t += 1
        chunk_plan.append((t_tiles - 1, 1))

        if isinstance(dma_pieces, (list, tuple)):
            bounds = sorted({min(bd, n_layers) for bd in dma_pieces} | {0, n_layers})
        else:
            bounds = [round(i * n_layers / dma_pieces)
                      for i in range(dma_pieces + 1)]

        def mlp_chunk(hT, t0, n_t, ppooled):
            cw = n_t * P
            h1 = h1pool.tile([P, HC * CWMAX], F32R, tag="h1")
            for m in range(HC):
                o1 = pmm1.tile([P, CWMAX], F32, tag="po")
                for k in range(FC):
                    nc.tensor.matmul(o1[:, 0:cw],
                                     w1t[:, k * H + m * P: k * H + (m + 1) * P],
                                     hT[:, k * CWMAX: k * CWMAX + cw],
                                     start=(k == 0), stop=(k == FC - 1))
                nc.scalar.activation(h1[:, m * CWMAX: m * CWMAX + cw],
                                     o1[:, 0:cw],
                                     mybir.ActivationFunctionType.Relu,
                                     bias=b1[:, m:m + 1], scale=1.0)
            for s in range(n_t):
                gi = t0 + s
                o2 = pmm2.tile([P, H], F32, tag="po")
                # bias enters the accumulation: ones.T @ (b2/128) == +b2 row
                nc.tensor.matmul(o2[:], ones[:], b2rep[:],
                                 start=True, stop=False)
                for m in range(HC):
                    nc.tensor.matmul(o2[:],
                                     h1[:, m * CWMAX + s * P: m * CWMAX + (s + 1) * P],
                                     w2t[:, m * H:(m + 1) * H],
                                     start=False, stop=(m == HC - 1))
                h2 = h2pool.tile([P, H], F32R, tag="h2")
                nc.scalar.activation(h2[:], o2[:],
                                     mybir.ActivationFunctionType.Relu)
                # per-sample masked pooling: msk tile is [token, 16] 0/1
                nc.tensor.matmul(ppooled[:], msk[:, gi * B:(gi + 1) * B], h2[:],
                                 start=(gi == 0), stop=(gi == t_tiles - 1),
                                 skip_group_check=True)

        def _body(_iv=None):
            ppooled = ppool.tile([B, H], F32, tag="pool")
            for (t0, n_t) in chunk_plan:
                hT = htpool.tile([P, FC * CWMAX], F32R, tag="hT")
                for s in range(n_t):
                    ti = t0 + s
                    tw = last_tw if ti == t_tiles - 1 else P
                    # the partial final tile runs an all-PE mix: PSUM rows
                    # >= tw come out zero, so no stale SBUF is ever read
                    n_pe = n_layers if tw < P else n_pe_layers
                    xt = xpool.tile([P, n_layers * F], F32R, tag="xt")
                    deng = nc.sync if (dma_engines == 1 or ti % 2 == 0) else nc.scalar
                    for lo, hi in zip(bounds[:-1], bounds[1:]):
                        deng.dma_start(
                            xt[0:tw, lo * F:hi * F],
                            xp_d[ti * P: ti * P + tw, lo * F:hi * F])
                    pm0 = pmix0.tile([P, 512], F32, tag="pm0")
                    pm1 = pmix1.tile([P, F - 512], F32, tag="pm1")
                    accd = None
                    for l in range(n_layers):
                        if l < n_pe:
                            se = seye[0:tw, l * P:(l + 1) * P]
                            st, sp = (l == 0), (l == n_pe - 1)
                            nc.tensor.matmul(pm0[:], se, xt[0:tw, l * F: l * F + 512],
                                             start=st, stop=sp)
                            nc.tensor.matmul(pm1[:], se, xt[0:tw, l * F + 512:(l + 1) * F],
                                             start=st, stop=sp)
                        else:
                            xf = xt[:, l * F:(l + 1) * F].bitcast(F32)
                            sc = svec[:, l:l + 1]
                            if accd is None:
                                accd = hpool.tile([P, F], F32, tag="accd")
                                nc.vector.tensor_scalar_mul(accd[:], xf, sc)
                            else:
                                nc.vector.scalar_tensor_tensor(
                                    accd[:], xf, sc, accd[:],
                                    op0=mybir.AluOpType.mult,
                                    op1=mybir.AluOpType.add)
                    # PSUM (+ DVE partial) -> SBUF mixed tile
                    h = hpool.tile([P, F], F32, tag="h")
                    if accd is None:
                        nc.scalar.copy(h[:, 0:512], pm0[:])
                        nc.scalar.copy(h[:, 512:F], pm1[:])
                    else:
                        nc.vector.scalar_tensor_tensor(
                            h[:, 0:512], pm0[:], 1.0, accd[:, 0:512],
                            op0=mybir.AluOpType.bypass, op1=mybir.AluOpType.add)
                        nc.vector.scalar_tensor_tensor(
                            h[:, 512:F], pm1[:], 1.0, accd[:, 512:F],
                            op0=mybir.AluOpType.bypass, op1=mybir.AluOpType.add)
                    # transpose 128x128 blocks into hT
                    for fc in range(FC):
                        pt = ptr.tile([P, P], F32, tag="po")
                        nc.tensor.transpose(pt[:], h[:, fc * P:(fc + 1) * P], ident[:])
                        dst = hT[:, fc * CWMAX + s * P: fc * CWMAX + (s + 1) * P]
                        if fc % 2 == 0:
                            nc.scalar.copy(dst, pt[:])
                        else:
                            nc.vector.tensor_copy(dst, pt[:])
                mlp_chunk(hT, t0, n_t, ppooled)
            nc.scalar.copy(pooled_sb[:], ppooled[:])

        if hw_loop_repeat is not None and hw_loop_repeat > 1:
            with tc.For_i(0, hw_loop_repeat, 1) as _i:
                _body(_i)
        else:
            _body()

        nc.sync.dma_start(out_d[:], pooled_sb[:])

    if split_waits:
        _split_excess_waits(nc, max_waits=1)
    return nc


class TileKernel:
    """TileContext + ExitStack in one `with`."""

    def __init__(self, nc):
        self.tc = tile.TileContext(nc)
        self.ctx = ExitStack()

    def __enter__(self):
        tc = self.tc.__enter__()
        self.ctx.__enter__()
        return tc, self.ctx

    def __exit__(self, *exc):
        self.ctx.__exit__(*exc)
        return self.tc.__exit__(*exc)


_PROGRAM_CACHE: dict[tuple, bass.Bass] = {}


def _get_program(n_layers: int, t_tiles: int, last_tw: int) -> bass.Bass:
    key = (n_layers, t_tiles, last_tw)
    if key not in _PROGRAM_CACHE:
        _PROGRAM_CACHE[key] = build_program(n_layers, t_tiles, last_tw)
    return _PROGRAM_CACHE[key]


def _softmax32(v: np.ndarray) -> np.ndarray:
    v = v.astype(np.float32)
    e = np.exp(v - v.max())
    return (e / e.sum()).astype(np.float32)


def _prep_in_maps(inputs: dict, n_layers: int):
    x = np.asarray(inputs["x"])
    lengths = np.asarray(inputs["lengths"]).astype(np.int64)

    t_tiles, last_tw, bs, ts, val = _plan_packing(lengths)
    cap = (t_tiles - 1) * P + last_tw

    # host-side prep of the small replicated operands
    s = (_softmax32(np.asarray(inputs["mixing_weights"]))
         * np.float32(np.asarray(inputs["gamma"]).reshape(-1)[0]))
    seye = np.zeros((P, n_layers * P), np.float32)
    for l in range(n_layers):
        seye[:, l * P:(l + 1) * P] = np.eye(P, dtype=np.float32) * s[l]
    svec = np.tile(s[:n_layers], (P, 1)).astype(np.float32)
    ident = np.eye(P, dtype=np.float32)

    W1 = np.asarray(inputs["W1"], np.float32)  # [H, F]
    W2 = np.asarray(inputs["W2"], np.float32)  # [H, H]
    w1t = np.ascontiguousarray(
        W1.T.reshape(FC, P, H).transpose(1, 0, 2).reshape(P, FC * H))
    w2t = np.ascontiguousarray(
        W2.T.reshape(HC, P, H).transpose(1, 0, 2).reshape(P, HC * H))
    b1p = np.ascontiguousarray(np.asarray(inputs["b1"], np.float32).reshape(HC, P).T)
    b2rep = np.tile(np.asarray(inputs["b2"], np.float32).reshape(1, H) / P, (P, 1))
    onesm = np.ones((P, P), np.float32)

    in_maps = []
    for c in range(N_CORES):
        xp = np.ascontiguousarray(
            x[bs[c], :n_layers, ts[c], :].reshape(cap, n_layers * F))
        mm = np.zeros((t_tiles * P, B), np.float32)
        mm[np.arange(cap), bs[c]] = val[c]
        mskp = np.ascontiguousarray(
            mm.reshape(t_tiles, P, B).transpose(1, 0, 2).reshape(P, t_tiles * B))
        in_maps.append({
            "xp": xp, "seye": seye, "svec": svec, "ident": ident,
            "w1t": w1t, "w2t": w2t, "b1": b1p, "b2rep": b2rep,
            "ones": onesm, "msk": mskp,
        })
    return in_maps, dict(t_tiles=t_tiles, last_tw=last_tw)


def _finish(pooled_parts, inputs):
    lengths = np.asarray(inputs["lengths"]).astype(np.float32)
    Wl = np.asarray(inputs["Wl"], np.float32)
    bl = np.asarray(inputs["bl"], np.float32)
    pooled = np.sum(np.stack(pooled_parts, 0), axis=0, dtype=np.float32)
    pooled = pooled / lengths[:, None]
    return (pooled @ Wl.T + bl).astype(np.float32)


def kernel(x, lengths, layer, gamma, mixing_weights, W1, b1, W2, b2, Wl, bl):
    n_layers = int(np.asarray(layer)) + 1
    assert 1 <= n_layers <= L

    inputs = dict(x=x, lengths=lengths, gamma=gamma,
                  mixing_weights=mixing_weights,
                  W1=W1, b1=b1, W2=W2, b2=b2, Wl=Wl, bl=bl)
    in_maps, pa = _prep_in_maps(inputs, n_layers)
    nc = _get_program(n_layers, pa["t_tiles"], pa["last_tw"])

    res = run_bass_kernel_spmd(nc, in_maps, list(range(N_CORES)))
    return _finish([res.results[c]["out"] for c in range(N_CORES)], inputs)


# revision 20
# speedup vs baseline: 1.2100x; 1.2100x over previous
"""Trainium2 Bass kernel for nn_CumulativeProbingDense.

Computation (see reference):
    h      = sum_l softmax(mixing_weights)[l] * x[:, l] * gamma   # [B, S, F]
    h1     = relu(h @ W1.T + b1)                                  # [B, S, H]
    h2     = relu(h1 @ W2.T + b2)                                 # [B, S, H]
    pooled = (h2 * mask).sum(S) / lengths                         # [B, H]
    logits = pooled @ Wl.T + bl                                   # [B, NL]

Sharding: tokens masked out by `lengths` (t >= lengths[b]) cannot affect the
output, so only the sum(lengths) valid tokens are processed.  The host packs
the valid (sample, token) pairs of ALL samples into one stream, split evenly
across the 8 cores (token-balanced data parallel).  Each core streams its
packed x slice ([t_tiles*128 tokens, L, F], token-major so DMA runs are
contiguous), computes the layer mix + MLP, and mask-matmul-pools per sample.
Host combines the per-core pooled partials and applies the tiny final linear.

Device pipeline per 128-token tile:
  - layer mix on TensorE: PSUM-accumulated matmuls with scaled-identity
    stationary; optionally a few trailing layers accumulate on the DVE
  - PE-transpose mixed tile into hT [feat part, token free]
  - mm1 (W1T stationary) -> relu+b1 on ScalarE -> h1 [hid part, token free]
  - mm2 with h1 chunks as STATIONARY and W2T moving -> h2 [token part, hid
    free]; b2 enters the same PSUM group as a ones x (b2/128) matmul
  - pooling: matmul with the per-sample 0/1 mask [token, 16] stationary,
    PSUM-accumulated over all tiles -> pooled [16, 256] per core
"""

import numpy as np

import concourse.bass as bass
import concourse.tile as tile
from concourse import mybir
from concourse.bass_utils import run_bass_kernel_spmd
from contextlib import ExitStack

F32 = mybir.dt.float32
F32R = mybir.dt.float32r

N_CORES = 8
B, L, S, F = 16, 13, 1024, 768
H, NL = 256, 7
P = 128                       # SBUF partitions
FC = F // P                   # feature chunks of 128
HC = H // P                   # hidden chunks of 128
CWMAX = 2 * P                 # token width of one MLP chunk

# matmul input dtype: float32r streams at 1 cycle/row (moving dim >= 256)
# vs plain float32's 4 cycles/row. fp32r rounds the operands (TF32-like).
MM_DT = F32R


def _split_excess_waits(nc, max_waits=1):
    """walrus (CoreV3) rejects instructions carrying more than a couple of
    sync waits (e.g. the TileContext exit drain). Hoist excess waits onto
    standalone NoOps inserted before the offending instruction."""
    n_fixed = 0
    for f in nc.m.functions:
        for bb in f.blocks:
            out, changed = [], False
            for inst in bb.instructions:
                si = getattr(inst, "sync_info", None)
                if si is not None and len(si.on_wait) > max_waits:
                    waits = list(si.on_wait)
                    for j, w in enumerate(waits[max_waits:]):
                        out.append(mybir.InstNoOp(
                            name=f"{inst.name}-wsplit{j}",
                            engine=inst.engine, ins=[], outs=[],
                            sync_info=mybir.SyncInfo(on_wait=[w], on_update=[]),
                        ))
                    inst.sync_info = mybir.SyncInfo(
                        on_wait=waits[:max_waits], on_update=list(si.on_update))
                    changed = True
                    n_fixed += 1
                out.append(inst)
            if changed:
                bb.instructions = out
    return n_fixed


def _plan_packing(lengths):
    """Token-balanced packing of all valid (sample, token) pairs onto cores.

    Returns (t_tiles, last_tw, b_idx [n_cores, cap], t_idx [n_cores, cap],
    valid [n_cores, cap]) with cap = (t_tiles-1)*128 + last_tw slots per
    core (the final token tile is partial); pad slots point at (0, 0) with
    valid=0."""
    lengths = np.asarray(lengths).astype(np.int64)
    total = int(lengths.sum())
    cap = max(1, -(-total // N_CORES))
    t_tiles = -(-cap // P)
    last_tw = cap - (t_tiles - 1) * P
    bs = np.repeat(np.arange(B, dtype=np.int64), lengths)
    ts = np.concatenate([np.arange(n, dtype=np.int64) for n in lengths])
    pad = N_CORES * cap - total
    bs = np.concatenate([bs, np.zeros(pad, np.int64)])
    ts = np.concatenate([ts, np.zeros(pad, np.int64)])
    val = np.concatenate([np.ones(total, np.float32), np.zeros(pad, np.float32)])
    return (t_tiles, last_tw, bs.reshape(N_CORES, cap),
            ts.reshape(N_CORES, cap), val.reshape(N_CORES, cap))


def build_program(n_layers: int, t_tiles: int, last_tw: int = P,
                  split_waits: bool = True,
                  hw_loop_repeat: int | None = None,
                  mix_dve_layers: int = 0,
                  x_bufs: int = 3,
                  dma_pieces=2,
                  dma_engines: int = 1,
                  unroll: int = 1) -> bass.Bass:
    # mix_dve_layers: trailing layers accumulated on the DVE (axpy) instead
    # of the TensorE, to balance PE vs DVE occupancy.
    n_pe_layers = n_layers - min(mix_dve_layers, n_layers - 1)
    cap = (t_tiles - 1) * P + last_tw
    nc = bass.Bass("TRN2", target_bir_lowering=False, debug=False, num_devices=1)

    xp_d = nc.dram_tensor("xp", [cap, n_layers * F], F32R, kind="ExternalInput").ap()
    seye_d = nc.dram_tensor("seye", [P, n_layers * P], F32R, kind="ExternalInput").ap()
    svec_d = nc.dram_tensor("svec", [P, n_layers], F32, kind="ExternalInput").ap()
    ident_d = nc.dram_tensor("ident", [P, P], F32, kind="ExternalInput").ap()
    w1t_d = nc.dram_tensor("w1t", [P, FC * H], F32R, kind="ExternalInput").ap()
    w2t_d = nc.dram_tensor("w2t", [P, HC * H], F32R, kind="ExternalInput").ap()
    b1_d = nc.dram_tensor("b1", [P, HC], F32, kind="ExternalInput").ap()
    b2rep_d = nc.dram_tensor("b2rep", [P, H], F32R, kind="ExternalInput").ap()
    ones_d = nc.dram_tensor("ones", [P, P], F32R, kind="ExternalInput").ap()
    msk_d = nc.dram_tensor("msk", [P, t_tiles * B], F32R, kind="ExternalInput").ap()
    out_d = nc.dram_tensor("out", [B, H], F32, kind="ExternalOutput").ap()

    with TileKernel(nc) as (tc, ctx):
        const = ctx.enter_context(tc.tile_pool(name="const", bufs=1))
        xpool = ctx.enter_context(tc.tile_pool(name="x", bufs=x_bufs))
        hpool = ctx.enter_context(tc.tile_pool(name="h", bufs=3))
        htpool = ctx.enter_context(tc.tile_pool(name="ht", bufs=2))
        h1pool = ctx.enter_context(tc.tile_pool(name="h1", bufs=2))
        h2pool = ctx.enter_context(tc.tile_pool(name="h2", bufs=2))
        pmix0 = ctx.enter_context(tc.tile_pool(name="pmix0", bufs=2, space="PSUM"))
        pmix1 = ctx.enter_context(tc.tile_pool(name="pmix1", bufs=2, space="PSUM"))
        pshared = ctx.enter_context(tc.tile_pool(name="pshared", bufs=3, space="PSUM"))
        ptr = pmm1 = pmm2 = pshared
        ppool = ctx.enter_context(tc.tile_pool(name="ppool", bufs=1, space="PSUM"))

        # ---- constants into SBUF via SWDGE (gpsimd), keeping the HWDGE
        # rings free for the x stream ----
        seye = const.tile([P, n_layers * P], F32R)
        nc.gpsimd.dma_start(seye[:], seye_d[:])
        svec = const.tile([P, n_layers], F32)
        nc.gpsimd.dma_start(svec[:], svec_d[:])
        ident = const.tile([P, P], F32)
        nc.gpsimd.dma_start(ident[:], ident_d[:])
        w1t = const.tile([P, FC * H], F32R)
        nc.gpsimd.dma_start(w1t[:], w1t_d[:])
        w2t = const.tile([P, HC * H], F32R)
        nc.gpsimd.dma_start(w2t[:], w2t_d[:])
        b1 = const.tile([P, HC], F32)
        nc.gpsimd.dma_start(b1[:], b1_d[:])
        b2rep = const.tile([P, H], F32R)
        nc.gpsimd.dma_start(b2rep[:], b2rep_d[:])
        ones = const.tile([P, P], F32R)
        nc.gpsimd.dma_start(ones[:], ones_d[:])
        msk = const.tile([P, t_tiles * B], F32R)
        nc.gpsimd.dma_start(msk[:], msk_d[:])

        pooled_sb = const.tile([B, H], F32)

        # MLP chunks of up to 2 token tiles (moving dim 256); the final
        # tile gets its own chunk so the post-DMA tail chain stays short
        chunk_plan = []
        rem = t_tiles - 1
        t = 0
        while t + 1 < rem:
            chunk_plan.append((t, 2))
            t += 2
        if t < rem:
            chunk_plan.append((t, 1))
            t += 1
        chunk_plan.append((t_tiles - 1, 1))

        if isinstance(dma_pieces, (list, tuple)):
            bounds = sorted({min(bd, n_layers) for bd in dma_pieces} | {0, n_layers})
        else:
            bounds = [round(i * n_layers / dma_pieces)
                      for i in range(dma_pieces + 1)]

        def mlp_chunk(hT, t0, n_t, ppooled):
            cw = n_t * P
            h1 = h1pool.tile([P, HC * CWMAX], F32R, tag="h1")
            for m in range(HC):
                o1 = pmm1.tile([P, CWMAX], F32, tag="po")
                for k in range(FC):
                    nc.tensor.matmul(o1[:, 0:cw],
                                     w1t[:, k * H + m * P: k * H + (m + 1) * P],
                                     hT[:, k * CWMAX: k * CWMAX + cw],
                                     start=(k == 0), stop=(k == FC - 1))
                nc.scalar.activation(h1[:, m * CWMAX: m * CWMAX + cw],
                                     o1[:, 0:cw],
                                     mybir.ActivationFunctionType.Relu,
                                     bias=b1[:, m:m + 1], scale=1.0)
            for s in range(n_t):
                gi = t0 + s
                o2 = pmm2.tile([P, H], F32, tag="po")
                # bias enters the accumulation: ones.T @ (b2/128) == +b2 row
                nc.tensor.matmul(o2[:], ones[:], b2rep[:],
                                 start=True, stop=False)
                for m in range(HC):
                    nc.tensor.matmul(o2[:],
                                     h1[:, m * CWMAX + s * P: m * CWMAX + (s + 1) * P],
                                     w2t[:, m * H:(m + 1) * H],
                                     start=False, stop=(m == HC - 1))
                h2 = h2pool.tile([P, H], F32R, tag="h2")
                nc.scalar.activation(h2[:], o2[:],
                                     mybir.ActivationFunctionType.Relu)
                # per-sample masked pooling: msk tile is [token, 16] 0/1
                nc.tensor.matmul(ppooled[:], msk[:, gi * B:(gi + 1) * B], h2[:],
                                 start=(gi == 0), stop=(gi == t_tiles - 1),
                                 skip_group_check=True)

        def _body(_iv=None):
            ppooled = ppool.tile([B, H], F32, tag="pool")
            for (t0, n_t) in chunk_plan:
                hT = htpool.tile([P, FC * CWMAX], F32R, tag="hT")
                for s in range(n_t):
                    ti = t0 + s
                    tw = last_tw if ti == t_tiles - 1 else P
                    # the partial final tile runs an all-PE mix: PSUM rows
                    # >= tw come out zero, so no stale SBUF is ever read
                    n_pe = n_layers if tw < P else n_pe_layers
                    xt = xpool.tile([P, n_layers * F], F32R, tag="xt")
                    deng = nc.sync if (dma_engines == 1 or ti % 2 == 0) else nc.scalar
                    for lo, hi in zip(bounds[:-1], bounds[1:]):
                        deng.dma_start(
                            xt[0:tw, lo * F:hi * F],
                            xp_d[ti * P: ti * P + tw, lo * F:hi * F])
                    pm0 = pmix0.tile([P, 512], F32, tag="pm0")
                    pm1 = pmix1.tile([P, F - 512], F32, tag="pm1")
                    accd = None
                    for l in range(n_layers):
                        if l < n_pe:
                            se = seye[0:tw, l * P:(l + 1) * P]
                            st, sp = (l == 0), (l == n_pe - 1)
                            nc.tensor.matmul(pm0[:], se, xt[0:tw, l * F: l * F + 512],
                                             start=st, stop=sp)
                            nc.tensor.matmul(pm1[:], se, xt[0:tw, l * F + 512:(l + 1) * F],
                                             start=st, stop=sp)
                        else:
                            xf = xt[:, l * F:(l + 1) * F].bitcast(F32)
                            sc = svec[:, l:l + 1]
                            if accd is None:
                                accd = hpool.tile([P, F], F32, tag="accd")
                                nc.vector.tensor_scalar_mul(accd[:], xf, sc)
                            else:
                                nc.vector.scalar_tensor_tensor(
                                    accd[:], xf, sc, accd[:],
                                    op0=mybir.AluOpType.mult,
                                    op1=mybir.AluOpType.add)
                    # PSUM (+ DVE partial) -> SBUF mixed tile
                    h = hpool.tile([P, F], F32, tag="h")
                    if accd is None:
                        nc.scalar.copy(h[:, 0:512], pm0[:])
                        nc.scalar.copy(h[:, 512:F], pm1[:])
                    else:
                        nc.vector.scalar_tensor_tensor(
                            h[:, 0:512], pm0[:], 1.0, accd[:, 0:512],
                            op0=mybir.AluOpType.bypass, op1=mybir.AluOpType.add)
                        nc.vector.scalar_tensor_tensor(
                            h[:, 512:F], pm1[:], 1.0, accd[:, 512:F],
                            op0=mybir.AluOpType.bypass, op1=mybir.AluOpType.add)
                    # transpose 128x128 blocks into hT
                    for fc in range(FC):
                        pt = ptr.tile([P, P], F32, tag="po")
                        nc.tensor.transpose(pt[:], h[:, fc * P:(fc + 1) * P], ident[:])
                        dst = hT[:, fc * CWMAX + s * P: fc * CWMAX + (s + 1) * P]
                        if fc % 2 == 0:
                            nc.scalar.copy(dst, pt[:])
                        else:
                            nc.vector.tensor_copy(dst, pt[:])
                mlp_chunk(hT, t0, n_t, ppooled)
            nc.scalar.copy(pooled_sb[:], ppooled[:])

        if hw_loop_repeat is not None and hw_loop_repeat > 1:
            # multiple unrolled passes per For_i iteration let the Tile
            # scheduler overlap one pass's tail with the next pass's DMA
            # (the loop body drains all engines at each iteration boundary)
            with tc.For_i(0, hw_loop_repeat, 1) as _i:
                for _u in range(unroll):
                    _body(_i)
        else:
            for _u in range(unroll):
                _body()

        nc.sync.dma_start(out_d[:], pooled_sb[:])

    if split_waits:
        _split_excess_waits(nc, max_waits=1)
    return nc


class TileKernel:
    """TileContext + ExitStack in one `with`."""

    def __init__(self, nc):
        self.tc = tile.TileContext(nc)
        self.ctx = ExitStack()

    def __enter__(self):
        tc = self.tc.__enter__()
        self.ctx.__enter__()
        return tc, self.ctx

    def __exit__(self, *exc):
        self.ctx.__exit__(*exc)
        return self.tc.__exit__(*exc)


_PROGRAM_CACHE: dict[tuple, bass.Bass] = {}


def _get_program(n_layers: int, t_tiles: int, last_tw: int) -> bass.Bass:
    key = (n_layers, t_tiles, last_tw)
    if key not in _PROGRAM_CACHE:
        _PROGRAM_CACHE[key] = build_program(n_layers, t_tiles, last_tw)
    return _PROGRAM_CACHE[key]


def _softmax32(v: np.ndarray) -> np.ndarray:
    v = v.astype(np.float32)
    e = np.exp(v - v.max())
    return (e / e.sum()).astype(np.float32)


def _prep_in_maps(inputs: dict, n_layers: int):
    x = np.asarray(inputs["x"])
    lengths = np.asarray(inputs["lengths"]).astype(np.int64)

    t_tiles, last_tw, bs, ts, val = _plan_packing(lengths)
    cap = (t_tiles - 1) * P + last_tw

    # host-side prep of the small replicated operands
    s = (_softmax32(np.asarray(inputs["mixing_weights"]))
         * np.float32(np.asarray(inputs["gamma"]).reshape(-1)[0]))
    seye = np.zeros((P, n_layers * P), np.float32)
    for l in range(n_layers):
        seye[:, l * P:(l + 1) * P] = np.eye(P, dtype=np.float32) * s[l]
    svec = np.tile(s[:n_layers], (P, 1)).astype(np.float32)
    ident = np.eye(P, dtype=np.float32)

    W1 = np.asarray(inputs["W1"], np.float32)  # [H, F]
    W2 = np.asarray(inputs["W2"], np.float32)  # [H, H]
    w1t = np.ascontiguousarray(
        W1.T.reshape(FC, P, H).transpose(1, 0, 2).reshape(P, FC * H))
    w2t = np.ascontiguousarray(
        W2.T.reshape(HC, P, H).transpose(1, 0, 2).reshape(P, HC * H))
    b1p = np.ascontiguousarray(np.asarray(inputs["b1"], np.float32).reshape(HC, P).T)
    b2rep = np.tile(np.asarray(inputs["b2"], np.float32).reshape(1, H) / P, (P, 1))
    onesm = np.ones((P, P), np.float32)

    in_maps = []
    for c in range(N_CORES):
        xp = np.ascontiguousarray(
            x[bs[c], :n_layers, ts[c], :].reshape(cap, n_layers * F))
        mm = np.zeros((t_tiles * P, B), np.float32)
        mm[np.arange(cap), bs[c]] = val[c]
        mskp = np.ascontiguousarray(
            mm.reshape(t_tiles, P, B).transpose(1, 0, 2).reshape(P, t_tiles * B))
        in_maps.append({
            "xp": xp, "seye": seye, "svec": svec, "ident": ident,
            "w1t": w1t, "w2t": w2t, "b1": b1p, "b2rep": b2rep,
            "ones": onesm, "msk": mskp,
        })
    return in_maps, dict(t_tiles=t_tiles, last_tw=last_tw)


def _finish(pooled_parts, inputs):
    lengths = np.asarray(inputs["lengths"]).astype(np.float32)
    Wl = np.asarray(inputs["Wl"], np.float32)
    bl = np.asarray(inputs["bl"], np.float32)
    pooled = np.sum(np.stack(pooled_parts, 0), axis=0, dtype=np.float32)
    pooled = pooled / lengths[:, None]
    return (pooled @ Wl.T + bl).astype(np.float32)


def kernel(x, lengths, layer, gamma, mixing_weights, W1, b1, W2, b2, Wl, bl):
    n_layers = int(np.asarray(layer)) + 1
    assert 1 <= n_layers <= L

    inputs = dict(x=x, lengths=lengths, gamma=gamma,
                  mixing_weights=mixing_weights,
                  W1=W1, b1=b1, W2=W2, b2=b2, Wl=Wl, bl=bl)
    in_maps, pa = _prep_in_maps(inputs, n_layers)
    nc = _get_program(n_layers, pa["t_tiles"], pa["last_tw"])

    res = run_bass_kernel_spmd(nc, in_maps, list(range(N_CORES)))
    return _finish([res.results[c]["out"] for c in range(N_CORES)], inputs)


# revision 29
# speedup vs baseline: 2.2338x; 1.8461x over previous
"""Trainium2 Bass kernel for nn_CumulativeProbingDense.

Computation (see reference):
    h      = sum_l softmax(mixing_weights)[l] * x[:, l] * gamma   # [B, S, F]
    h1     = relu(h @ W1.T + b1)                                  # [B, S, H]
    h2     = relu(h1 @ W2.T + b2)                                 # [B, S, H]
    pooled = (h2 * mask).sum(S) / lengths                         # [B, H]
    logits = pooled @ Wl.T + bl                                   # [B, NL]

Sharding: tokens masked out by `lengths` (t >= lengths[b]) cannot affect the
output, so only the sum(lengths) valid tokens are processed.  The host packs
the valid (sample, token) pairs of ALL samples into one stream, split evenly
across the 8 cores (token-balanced data parallel).  Each core streams its
packed x slice ([t_tiles*128 tokens, L, F], token-major so DMA runs are
contiguous), computes the layer mix + MLP, and mask-matmul-pools per sample.
Host combines the per-core pooled partials and applies the tiny final linear.

Device pipeline per 128-token tile:
  - layer mix on TensorE: PSUM-accumulated matmuls with scaled-identity
    stationary; optionally a few trailing layers accumulate on the DVE
  - PE-transpose mixed tile into hT [feat part, token free]
  - mm1 (W1T stationary) -> relu+b1 on ScalarE -> h1 [hid part, token free]
  - mm2 with h1 chunks as STATIONARY and W2T moving -> h2 [token part, hid
    free]; b2 enters the same PSUM group as a ones x (b2/128) matmul
  - pooling: matmul with the per-sample 0/1 mask [token, 16] stationary,
    PSUM-accumulated over all tiles -> pooled [16, 256] per core
"""

import numpy as np

import concourse.bass as bass
import concourse.tile as tile
from concourse import mybir
from concourse.bass_utils import run_bass_kernel_spmd
from contextlib import ExitStack

F32 = mybir.dt.float32
F32R = mybir.dt.float32r
BF16 = mybir.dt.bfloat16

# stream x as bf16: the host gather already copies x, so staging the copy at
# bf16 halves HBM traffic; all FLOPs stay on device (PE accumulates fp32).
# Measured end-to-end rel err vs the fp32 reference stays ~1e-3 (gate 2e-2).
X_BF16 = True

N_CORES = 8
B, L, S, F = 16, 13, 1024, 768
H, NL = 256, 7
P = 128                       # SBUF partitions
FC = F // P                   # feature chunks of 128
HC = H // P                   # hidden chunks of 128
CWMAX = 2 * P                 # token width of one MLP chunk

# matmul input dtype: float32r streams at 1 cycle/row (moving dim >= 256)
# vs plain float32's 4 cycles/row. fp32r rounds the operands (TF32-like).
MM_DT = F32R


def _split_excess_waits(nc, max_waits=1):
    """walrus (CoreV3) rejects instructions carrying more than a couple of
    sync waits (e.g. the TileContext exit drain). Hoist excess waits onto
    standalone NoOps inserted before the offending instruction."""
    n_fixed = 0
    for f in nc.m.functions:
        for bb in f.blocks:
            out, changed = [], False
            for inst in bb.instructions:
                si = getattr(inst, "sync_info", None)
                if si is not None and len(si.on_wait) > max_waits:
                    waits = list(si.on_wait)
                    for j, w in enumerate(waits[max_waits:]):
                        out.append(mybir.InstNoOp(
                            name=f"{inst.name}-wsplit{j}",
                            engine=inst.engine, ins=[], outs=[],
                            sync_info=mybir.SyncInfo(on_wait=[w], on_update=[]),
                        ))
                    inst.sync_info = mybir.SyncInfo(
                        on_wait=waits[:max_waits], on_update=list(si.on_update))
                    changed = True
                    n_fixed += 1
                out.append(inst)
            if changed:
                bb.instructions = out
    return n_fixed


def _plan_packing(lengths):
    """Token-balanced packing of all valid (sample, token) pairs onto cores.

    Returns (t_tiles, last_tw, b_idx [n_cores, cap], t_idx [n_cores, cap],
    valid [n_cores, cap]) with cap = (t_tiles-1)*128 + last_tw slots per
    core (the final token tile is partial); pad slots point at (0, 0) with
    valid=0."""
    lengths = np.asarray(lengths).astype(np.int64)
    total = int(lengths.sum())
    cap = max(1, -(-total // N_CORES))
    t_tiles = -(-cap // P)
    last_tw = cap - (t_tiles - 1) * P
    bs = np.repeat(np.arange(B, dtype=np.int64), lengths)
    ts = np.concatenate([np.arange(n, dtype=np.int64) for n in lengths])
    pad = N_CORES * cap - total
    bs = np.concatenate([bs, np.zeros(pad, np.int64)])
    ts = np.concatenate([ts, np.zeros(pad, np.int64)])
    val = np.concatenate([np.ones(total, np.float32), np.zeros(pad, np.float32)])
    return (t_tiles, last_tw, bs.reshape(N_CORES, cap),
            ts.reshape(N_CORES, cap), val.reshape(N_CORES, cap))


def build_program(n_layers: int, t_tiles: int, last_tw: int = P,
                  split_waits: bool = True,
                  hw_loop_repeat: int | None = None,
                  mix_dve_layers: int = 0,
                  x_bufs: int = 3,
                  dma_pieces=2,
                  dma_engines: int = 1,
                  unroll: int = 1,
                  x_bf16: bool = X_BF16) -> bass.Bass:
    # mix_dve_layers: trailing layers accumulated on the DVE (axpy) instead
    # of the TensorE, to balance PE vs DVE occupancy.
    if x_bf16:
        mix_dve_layers = 0      # DVE mix path assumes 4-byte x (bitcast)
    n_pe_layers = n_layers - min(mix_dve_layers, n_layers - 1)
    cap = (t_tiles - 1) * P + last_tw
    nc = bass.Bass("TRN2", target_bir_lowering=False, debug=False, num_devices=1)

    XDT = BF16 if x_bf16 else F32R
    xp_d = nc.dram_tensor("xp", [cap, n_layers * F], XDT, kind="ExternalInput").ap()
    seye_d = nc.dram_tensor("seye", [P, n_layers * P], XDT, kind="ExternalInput").ap()
    svec_d = nc.dram_tensor("svec", [P, n_layers], F32, kind="ExternalInput").ap()
    ident_d = nc.dram_tensor("ident", [P, P], F32, kind="ExternalInput").ap()
    w1t_d = nc.dram_tensor("w1t", [P, FC * H], F32R, kind="ExternalInput").ap()
    w2t_d = nc.dram_tensor("w2t", [P, HC * H], F32R, kind="ExternalInput").ap()
    b1_d = nc.dram_tensor("b1", [P, HC], F32, kind="ExternalInput").ap()
    b2rep_d = nc.dram_tensor("b2rep", [P, H], F32R, kind="ExternalInput").ap()
    ones_d = nc.dram_tensor("ones", [P, P], F32R, kind="ExternalInput").ap()
    msk_d = nc.dram_tensor("msk", [P, t_tiles * B], F32R, kind="ExternalInput").ap()
    out_d = nc.dram_tensor("out", [B, H], F32, kind="ExternalOutput").ap()

    with TileKernel(nc) as (tc, ctx):
        const = ctx.enter_context(tc.tile_pool(name="const", bufs=1))
        xpool = ctx.enter_context(tc.tile_pool(name="x", bufs=x_bufs))
        hpool = ctx.enter_context(tc.tile_pool(name="h", bufs=3))
        htpool = ctx.enter_context(tc.tile_pool(name="ht", bufs=2))
        h1pool = ctx.enter_context(tc.tile_pool(name="h1", bufs=2))
        h2pool = ctx.enter_context(tc.tile_pool(name="h2", bufs=2))
        pmix0 = ctx.enter_context(tc.tile_pool(name="pmix0", bufs=2, space="PSUM"))
        pmix1 = ctx.enter_context(tc.tile_pool(name="pmix1", bufs=2, space="PSUM"))
        pshared = ctx.enter_context(tc.tile_pool(name="pshared", bufs=3, space="PSUM"))
        ptr = pmm1 = pmm2 = pshared
        ppool = ctx.enter_context(tc.tile_pool(name="ppool", bufs=1, space="PSUM"))

        # ---- constants into SBUF via SWDGE (gpsimd), keeping the HWDGE
        # rings free for the x stream ----
        seye = const.tile([P, n_layers * P], XDT)
        nc.gpsimd.dma_start(seye[:], seye_d[:])
        svec = const.tile([P, n_layers], F32)
        nc.gpsimd.dma_start(svec[:], svec_d[:])
        ident = const.tile([P, P], F32)
        nc.gpsimd.dma_start(ident[:], ident_d[:])
        w1t = const.tile([P, FC * H], F32R)
        nc.gpsimd.dma_start(w1t[:], w1t_d[:])
        w2t = const.tile([P, HC * H], F32R)
        nc.gpsimd.dma_start(w2t[:], w2t_d[:])
        b1 = const.tile([P, HC], F32)
        nc.gpsimd.dma_start(b1[:], b1_d[:])
        b2rep = const.tile([P, H], F32R)
        nc.gpsimd.dma_start(b2rep[:], b2rep_d[:])
        ones = const.tile([P, P], F32R)
        nc.gpsimd.dma_start(ones[:], ones_d[:])
        msk = const.tile([P, t_tiles * B], F32R)
        nc.gpsimd.dma_start(msk[:], msk_d[:])

        pooled_sb = const.tile([B, H], F32)

        # MLP chunks of up to 2 token tiles (moving dim 256); the final
        # tile gets its own chunk so the post-DMA tail chain stays short
        chunk_plan = []
        rem = t_tiles - 1
        t = 0
        while t + 1 < rem:
            chunk_plan.append((t, 2))
            t += 2
        if t < rem:
            chunk_plan.append((t, 1))
            t += 1
        chunk_plan.append((t_tiles - 1, 1))

        if isinstance(dma_pieces, (list, tuple)):
            bounds = sorted({min(bd, n_layers) for bd in dma_pieces} | {0, n_layers})
        else:
            bounds = [round(i * n_layers / dma_pieces)
                      for i in range(dma_pieces + 1)]

        def mlp_chunk(hT, t0, n_t, ppooled):
            cw = n_t * P
            h1 = h1pool.tile([P, HC * CWMAX], F32R, tag="h1")
            for m in range(HC):
                o1 = pmm1.tile([P, CWMAX], F32, tag="po")
                for k in range(FC):
                    nc.tensor.matmul(o1[:, 0:cw],
                                     w1t[:, k * H + m * P: k * H + (m + 1) * P],
                                     hT[:, k * CWMAX: k * CWMAX + cw],
                                     start=(k == 0), stop=(k == FC - 1))
                nc.scalar.activation(h1[:, m * CWMAX: m * CWMAX + cw],
                                     o1[:, 0:cw],
                                     mybir.ActivationFunctionType.Relu,
                                     bias=b1[:, m:m + 1], scale=1.0)
            for s in range(n_t):
                gi = t0 + s
                o2 = pmm2.tile([P, H], F32, tag="po")
                # bias enters the accumulation: ones.T @ (b2/128) == +b2 row
                nc.tensor.matmul(o2[:], ones[:], b2rep[:],
                                 start=True, stop=False)
                for m in range(HC):
                    nc.tensor.matmul(o2[:],
                                     h1[:, m * CWMAX + s * P: m * CWMAX + (s + 1) * P],
                                     w2t[:, m * H:(m + 1) * H],
                                     start=False, stop=(m == HC - 1))
                h2 = h2pool.tile([P, H], F32R, tag="h2")
                nc.scalar.activation(h2[:], o2[:],
                                     mybir.ActivationFunctionType.Relu)
                # per-sample masked pooling: msk tile is [token, 16] 0/1
                nc.tensor.matmul(ppooled[:], msk[:, gi * B:(gi + 1) * B], h2[:],
                                 start=(gi == 0), stop=(gi == t_tiles - 1),
                                 skip_group_check=True)

        def _body(_iv=None):
            ppooled = ppool.tile([B, H], F32, tag="pool")
            for (t0, n_t) in chunk_plan:
                hT = htpool.tile([P, FC * CWMAX], F32R, tag="hT")
                for s in range(n_t):
                    ti = t0 + s
                    tw = last_tw if ti == t_tiles - 1 else P
                    # the partial final tile runs an all-PE mix: PSUM rows
                    # >= tw come out zero, so no stale SBUF is ever read
                    n_pe = n_layers if tw < P else n_pe_layers
                    xt = xpool.tile([P, n_layers * F], XDT, tag="xt")
                    deng = nc.sync if (dma_engines == 1 or ti % 2 == 0) else nc.scalar
                    for lo, hi in zip(bounds[:-1], bounds[1:]):
                        deng.dma_start(
                            xt[0:tw, lo * F:hi * F],
                            xp_d[ti * P: ti * P + tw, lo * F:hi * F])
                    pm0 = pmix0.tile([P, 512], F32, tag="pm0")
                    pm1 = pmix1.tile([P, F - 512], F32, tag="pm1")
                    accd = None
                    for l in range(n_layers):
                        if l < n_pe:
                            se = seye[0:tw, l * P:(l + 1) * P]
                            st, sp = (l == 0), (l == n_pe - 1)
                            nc.tensor.matmul(pm0[:], se, xt[0:tw, l * F: l * F + 512],
                                             start=st, stop=sp)
                            nc.tensor.matmul(pm1[:], se, xt[0:tw, l * F + 512:(l + 1) * F],
                                             start=st, stop=sp)
                        else:
                            xf = xt[:, l * F:(l + 1) * F].bitcast(F32)
                            sc = svec[:, l:l + 1]
                            if accd is None:
                                accd = hpool.tile([P, F], F32, tag="accd")
                                nc.vector.tensor_scalar_mul(accd[:], xf, sc)
                            else:
                                nc.vector.scalar_tensor_tensor(
                                    accd[:], xf, sc, accd[:],
                                    op0=mybir.AluOpType.mult,
                                    op1=mybir.AluOpType.add)
                    # PSUM (+ DVE partial) -> SBUF mixed tile
                    h = hpool.tile([P, F], F32, tag="h")
                    if accd is None:
                        nc.scalar.copy(h[:, 0:512], pm0[:])
                        nc.scalar.copy(h[:, 512:F], pm1[:])
                    else:
                        nc.vector.scalar_tensor_tensor(
                            h[:, 0:512], pm0[:], 1.0, accd[:, 0:512],
                            op0=mybir.AluOpType.bypass, op1=mybir.AluOpType.add)
                        nc.vector.scalar_tensor_tensor(
                            h[:, 512:F], pm1[:], 1.0, accd[:, 512:F],
                            op0=mybir.AluOpType.bypass, op1=mybir.AluOpType.add)
                    # transpose 128x128 blocks into hT
                    for fc in range(FC):
                        pt = ptr.tile([P, P], F32, tag="po")
                        nc.tensor.transpose(pt[:], h[:, fc * P:(fc + 1) * P], ident[:])
                        dst = hT[:, fc * CWMAX + s * P: fc * CWMAX + (s + 1) * P]
                        if fc % 2 == 0:
                            nc.scalar.copy(dst, pt[:])
                        else:
                            nc.vector.tensor_copy(dst, pt[:])
                mlp_chunk(hT, t0, n_t, ppooled)
            nc.scalar.copy(pooled_sb[:], ppooled[:])

        if hw_loop_repeat is not None and hw_loop_repeat > 1:
            # multiple unrolled passes per For_i iteration let the Tile
            # scheduler overlap one pass's tail with the next pass's DMA
            # (the loop body drains all engines at each iteration boundary)
            with tc.For_i(0, hw_loop_repeat, 1) as _i:
                for _u in range(unroll):
                    _body(_i)
        else:
            for _u in range(unroll):
                _body()

        nc.sync.dma_start(out_d[:], pooled_sb[:])

    if split_waits:
        _split_excess_waits(nc, max_waits=1)
    return nc


class TileKernel:
    """TileContext + ExitStack in one `with`."""

    def __init__(self, nc):
        self.tc = tile.TileContext(nc)
        self.ctx = ExitStack()

    def __enter__(self):
        tc = self.tc.__enter__()
        self.ctx.__enter__()
        return tc, self.ctx

    def __exit__(self, *exc):
        self.ctx.__exit__(*exc)
        return self.tc.__exit__(*exc)


_PROGRAM_CACHE: dict[tuple, bass.Bass] = {}


def _get_program(n_layers: int, t_tiles: int, last_tw: int) -> bass.Bass:
    key = (n_layers, t_tiles, last_tw)
    if key not in _PROGRAM_CACHE:
        _PROGRAM_CACHE[key] = build_program(n_layers, t_tiles, last_tw)
    return _PROGRAM_CACHE[key]


def _softmax32(v: np.ndarray) -> np.ndarray:
    v = v.astype(np.float32)
    e = np.exp(v - v.max())
    return (e / e.sum()).astype(np.float32)


def _prep_in_maps(inputs: dict, n_layers: int):
    x = np.asarray(inputs["x"])
    lengths = np.asarray(inputs["lengths"]).astype(np.int64)

    t_tiles, last_tw, bs, ts, val = _plan_packing(lengths)
    cap = (t_tiles - 1) * P + last_tw

    # host-side prep of the small replicated operands
    s = (_softmax32(np.asarray(inputs["mixing_weights"]))
         * np.float32(np.asarray(inputs["gamma"]).reshape(-1)[0]))
    seye = np.zeros((P, n_layers * P), np.float32)
    for l in range(n_layers):
        seye[:, l * P:(l + 1) * P] = np.eye(P, dtype=np.float32) * s[l]
    if X_BF16:
        import ml_dtypes
        seye = seye.astype(ml_dtypes.bfloat16)
    svec = np.tile(s[:n_layers], (P, 1)).astype(np.float32)
    ident = np.eye(P, dtype=np.float32)

    W1 = np.asarray(inputs["W1"], np.float32)  # [H, F]
    W2 = np.asarray(inputs["W2"], np.float32)  # [H, H]
    w1t = np.ascontiguousarray(
        W1.T.reshape(FC, P, H).transpose(1, 0, 2).reshape(P, FC * H))
    w2t = np.ascontiguousarray(
        W2.T.reshape(HC, P, H).transpose(1, 0, 2).reshape(P, HC * H))
    b1p = np.ascontiguousarray(np.asarray(inputs["b1"], np.float32).reshape(HC, P).T)
    b2rep = np.tile(np.asarray(inputs["b2"], np.float32).reshape(1, H) / P, (P, 1))
    onesm = np.ones((P, P), np.float32)

    in_maps = []
    for c in range(N_CORES):
        xp = np.ascontiguousarray(
            x[bs[c], :n_layers, ts[c], :].reshape(cap, n_layers * F))
        if X_BF16:
            import ml_dtypes
            xp = xp.astype(ml_dtypes.bfloat16)
        mm = np.zeros((t_tiles * P, B), np.float32)
        mm[np.arange(cap), bs[c]] = val[c]
        mskp = np.ascontiguousarray(
            mm.reshape(t_tiles, P, B).transpose(1, 0, 2).reshape(P, t_tiles * B))
        in_maps.append({
            "xp": xp, "seye": seye, "svec": svec, "ident": ident,
            "w1t": w1t, "w2t": w2t, "b1": b1p, "b2rep": b2rep,
            "ones": onesm, "msk": mskp,
        })
    return in_maps, dict(t_tiles=t_tiles, last_tw=last_tw)


def _finish(pooled_parts, inputs):
    lengths = np.asarray(inputs["lengths"]).astype(np.float32)
    Wl = np.asarray(inputs["Wl"], np.float32)
    bl = np.asarray(inputs["bl"], np.float32)
    pooled = np.sum(np.stack(pooled_parts, 0), axis=0, dtype=np.float32)
    pooled = pooled / lengths[:, None]
    return (pooled @ Wl.T + bl).astype(np.float32)


def kernel(x, lengths, layer, gamma, mixing_weights, W1, b1, W2, b2, Wl, bl):
    n_layers = int(np.asarray(layer)) + 1
    assert 1 <= n_layers <= L

    inputs = dict(x=x, lengths=lengths, gamma=gamma,
                  mixing_weights=mixing_weights,
                  W1=W1, b1=b1, W2=W2, b2=b2, Wl=Wl, bl=bl)
    in_maps, pa = _prep_in_maps(inputs, n_layers)
    nc = _get_program(n_layers, pa["t_tiles"], pa["last_tw"])

    res = run_bass_kernel_spmd(nc, in_maps, list(range(N_CORES)))
    return _finish([res.results[c]["out"] for c in range(N_CORES)], inputs)


# revision 34
# speedup vs baseline: 2.3204x; 1.0388x over previous
"""Trainium2 Bass kernel for nn_CumulativeProbingDense.

Computation (see reference):
    h      = sum_l softmax(mixing_weights)[l] * x[:, l] * gamma   # [B, S, F]
    h1     = relu(h @ W1.T + b1)                                  # [B, S, H]
    h2     = relu(h1 @ W2.T + b2)                                 # [B, S, H]
    pooled = (h2 * mask).sum(S) / lengths                         # [B, H]
    logits = pooled @ Wl.T + bl                                   # [B, NL]

Sharding: tokens masked out by `lengths` (t >= lengths[b]) cannot affect the
output, so only the sum(lengths) valid tokens are processed.  The host packs
the valid (sample, token) pairs of ALL samples into one stream, split evenly
across the 8 cores (token-balanced data parallel).  Each core streams its
packed x slice ([t_tiles*128 tokens, L, F], token-major so DMA runs are
contiguous), computes the layer mix + MLP, and mask-matmul-pools per sample.
Host combines the per-core pooled partials and applies the tiny final linear.

Device pipeline per 128-token tile:
  - layer mix on TensorE: PSUM-accumulated matmuls with scaled-identity
    stationary; optionally a few trailing layers accumulate on the DVE
  - PE-transpose mixed tile into hT [feat part, token free]
  - mm1 (W1T stationary) -> relu+b1 on ScalarE -> h1 [hid part, token free]
  - mm2 with h1 chunks as STATIONARY and W2T moving -> h2 [token part, hid
    free]; b2 enters the same PSUM group as a ones x (b2/128) matmul
  - pooling: matmul with the per-sample 0/1 mask [token, 16] stationary,
    PSUM-accumulated over all tiles -> pooled [16, 256] per core
"""

import numpy as np

import concourse.bass as bass
import concourse.tile as tile
from concourse import mybir
from concourse.bass_utils import run_bass_kernel_spmd
from contextlib import ExitStack

F32 = mybir.dt.float32
F32R = mybir.dt.float32r
BF16 = mybir.dt.bfloat16

# stream x as bf16: the host gather already copies x, so staging the copy at
# bf16 halves HBM traffic; all FLOPs stay on device (PE accumulates fp32).
# Measured end-to-end rel err vs the fp32 reference stays ~1e-3 (gate 2e-2).
X_BF16 = True

N_CORES = 8
B, L, S, F = 16, 13, 1024, 768
H, NL = 256, 7
P = 128                       # SBUF partitions
FC = F // P                   # feature chunks of 128
HC = H // P                   # hidden chunks of 128
CWMAX = 2 * P                 # token width of one MLP chunk

# matmul input dtype: float32r streams at 1 cycle/row (moving dim >= 256)
# vs plain float32's 4 cycles/row. fp32r rounds the operands (TF32-like).
MM_DT = F32R


def _split_excess_waits(nc, max_waits=1):
    """walrus (CoreV3) rejects instructions carrying more than a couple of
    sync waits (e.g. the TileContext exit drain). Hoist excess waits onto
    standalone NoOps inserted before the offending instruction."""
    n_fixed = 0
    for f in nc.m.functions:
        for bb in f.blocks:
            out, changed = [], False
            for inst in bb.instructions:
                si = getattr(inst, "sync_info", None)
                if si is not None and len(si.on_wait) > max_waits:
                    waits = list(si.on_wait)
                    for j, w in enumerate(waits[max_waits:]):
                        out.append(mybir.InstNoOp(
                            name=f"{inst.name}-wsplit{j}",
                            engine=inst.engine, ins=[], outs=[],
                            sync_info=mybir.SyncInfo(on_wait=[w], on_update=[]),
                        ))
                    inst.sync_info = mybir.SyncInfo(
                        on_wait=waits[:max_waits], on_update=list(si.on_update))
                    changed = True
                    n_fixed += 1
                out.append(inst)
            if changed:
                bb.instructions = out
    return n_fixed


def _plan_packing(lengths):
    """Token-balanced packing of all valid (sample, token) pairs onto cores.

    Returns (t_tiles, last_tw, b_idx [n_cores, cap], t_idx [n_cores, cap],
    valid [n_cores, cap]) with cap = (t_tiles-1)*128 + last_tw slots per
    core (the final token tile is partial); pad slots point at (0, 0) with
    valid=0."""
    lengths = np.asarray(lengths).astype(np.int64)
    total = int(lengths.sum())
    cap = max(1, -(-total // N_CORES))
    t_tiles = -(-cap // P)
    last_tw = cap - (t_tiles - 1) * P
    bs = np.repeat(np.arange(B, dtype=np.int64), lengths)
    ts = np.concatenate([np.arange(n, dtype=np.int64) for n in lengths])
    pad = N_CORES * cap - total
    bs = np.concatenate([bs, np.zeros(pad, np.int64)])
    ts = np.concatenate([ts, np.zeros(pad, np.int64)])
    val = np.concatenate([np.ones(total, np.float32), np.zeros(pad, np.float32)])
    return (t_tiles, last_tw, bs.reshape(N_CORES, cap),
            ts.reshape(N_CORES, cap), val.reshape(N_CORES, cap))


def build_program(n_layers: int, t_tiles: int, last_tw: int = P,
                  split_waits: bool = True,
                  hw_loop_repeat: int | None = None,
                  mix_dve_layers: int = 0,
                  x_bufs: int = 3,
                  # small-first split: a 3-layer piece lands early so the mix
                  # starts ~2.5us sooner per tile (measured win over 50/50)
                  dma_pieces=(3,),
                  dma_engines: int = 1,
                  unroll: int = 1,
                  x_bf16: bool = X_BF16,
                  last_tile_cut: int = 0,
                  deep_bufs: int = 0) -> bass.Bass:
    # mix_dve_layers: trailing layers accumulated on the DVE (axpy) instead
    # of the TensorE, to balance PE vs DVE occupancy.
    if x_bf16:
        mix_dve_layers = 0      # DVE mix path assumes 4-byte x (bitcast)
    n_pe_layers = n_layers - min(mix_dve_layers, n_layers - 1)
    cap = (t_tiles - 1) * P + last_tw
    nc = bass.Bass("TRN2", target_bir_lowering=False, debug=False, num_devices=1)

    XDT = BF16 if x_bf16 else F32R
    xp_d = nc.dram_tensor("xp", [cap, n_layers * F], XDT, kind="ExternalInput").ap()
    seye_d = nc.dram_tensor("seye", [P, n_layers * P], XDT, kind="ExternalInput").ap()
    svec_d = nc.dram_tensor("svec", [P, n_layers], F32, kind="ExternalInput").ap()
    ident_d = nc.dram_tensor("ident", [P, P], F32, kind="ExternalInput").ap()
    w1t_d = nc.dram_tensor("w1t", [P, FC * H], F32R, kind="ExternalInput").ap()
    w2t_d = nc.dram_tensor("w2t", [P, HC * H], F32R, kind="ExternalInput").ap()
    b1_d = nc.dram_tensor("b1", [P, HC], F32, kind="ExternalInput").ap()
    b2rep_d = nc.dram_tensor("b2rep", [P, H], F32R, kind="ExternalInput").ap()
    ones_d = nc.dram_tensor("ones", [P, P], F32R, kind="ExternalInput").ap()
    msk_d = nc.dram_tensor("msk", [P, t_tiles * B], F32R, kind="ExternalInput").ap()
    out_d = nc.dram_tensor("out", [B, H], F32, kind="ExternalOutput").ap()

    with TileKernel(nc) as (tc, ctx):
        const = ctx.enter_context(tc.tile_pool(name="const", bufs=1))
        xpool = ctx.enter_context(tc.tile_pool(name="x", bufs=x_bufs))
        hpool = ctx.enter_context(tc.tile_pool(name="h", bufs=3 + deep_bufs))
        htpool = ctx.enter_context(tc.tile_pool(name="ht", bufs=2 + deep_bufs))
        h1pool = ctx.enter_context(tc.tile_pool(name="h1", bufs=2 + deep_bufs))
        h2pool = ctx.enter_context(tc.tile_pool(name="h2", bufs=2 + 2 * deep_bufs))
        pmix0 = ctx.enter_context(tc.tile_pool(name="pmix0", bufs=2, space="PSUM"))
        pmix1 = ctx.enter_context(tc.tile_pool(name="pmix1", bufs=2, space="PSUM"))
        pshared = ctx.enter_context(tc.tile_pool(name="pshared", bufs=3, space="PSUM"))
        ptr = pmm1 = pmm2 = pshared
        ppool = ctx.enter_context(tc.tile_pool(name="ppool", bufs=1, space="PSUM"))

        # ---- constants into SBUF via SWDGE (gpsimd), keeping the HWDGE
        # rings free for the x stream ----
        seye = const.tile([P, n_layers * P], XDT)
        nc.gpsimd.dma_start(seye[:], seye_d[:])
        svec = const.tile([P, n_layers], F32)
        nc.gpsimd.dma_start(svec[:], svec_d[:])
        ident = const.tile([P, P], F32)
        nc.gpsimd.dma_start(ident[:], ident_d[:])
        w1t = const.tile([P, FC * H], F32R)
        nc.gpsimd.dma_start(w1t[:], w1t_d[:])
        w2t = const.tile([P, HC * H], F32R)
        nc.gpsimd.dma_start(w2t[:], w2t_d[:])
        b1 = const.tile([P, HC], F32)
        nc.gpsimd.dma_start(b1[:], b1_d[:])
        b2rep = const.tile([P, H], F32R)
        nc.gpsimd.dma_start(b2rep[:], b2rep_d[:])
        ones = const.tile([P, P], F32R)
        nc.gpsimd.dma_start(ones[:], ones_d[:])
        msk = const.tile([P, t_tiles * B], F32R)
        nc.gpsimd.dma_start(msk[:], msk_d[:])

        pooled_sb = const.tile([B, H], F32)

        # MLP chunks of up to 2 token tiles (moving dim 256); the final
        # tile gets its own chunk so the post-DMA tail chain stays short
        chunk_plan = []
        rem = t_tiles - 1
        t = 0
        while t + 1 < rem:
            chunk_plan.append((t, 2))
            t += 2
        if t < rem:
            chunk_plan.append((t, 1))
            t += 1
        chunk_plan.append((t_tiles - 1, 1))

        if isinstance(dma_pieces, (list, tuple)):
            bounds = sorted({min(bd, n_layers) for bd in dma_pieces} | {0, n_layers})
        else:
            bounds = [round(i * n_layers / dma_pieces)
                      for i in range(dma_pieces + 1)]

        def mlp_chunk(hT, t0, n_t, ppooled):
            cw = n_t * P
            h1 = h1pool.tile([P, HC * CWMAX], F32R, tag="h1")
            for m in range(HC):
                o1 = pmm1.tile([P, CWMAX], F32, tag="po")
                for k in range(FC):
                    nc.tensor.matmul(o1[:, 0:cw],
                                     w1t[:, k * H + m * P: k * H + (m + 1) * P],
                                     hT[:, k * CWMAX: k * CWMAX + cw],
                                     start=(k == 0), stop=(k == FC - 1))
                nc.scalar.activation(h1[:, m * CWMAX: m * CWMAX + cw],
                                     o1[:, 0:cw],
                                     mybir.ActivationFunctionType.Relu,
                                     bias=b1[:, m:m + 1], scale=1.0)
            for s in range(n_t):
                gi = t0 + s
                o2 = pmm2.tile([P, H], F32, tag="po")
                # bias enters the accumulation: ones.T @ (b2/128) == +b2 row
                nc.tensor.matmul(o2[:], ones[:], b2rep[:],
                                 start=True, stop=False)
                for m in range(HC):
                    nc.tensor.matmul(o2[:],
                                     h1[:, m * CWMAX + s * P: m * CWMAX + (s + 1) * P],
                                     w2t[:, m * H:(m + 1) * H],
                                     start=False, stop=(m == HC - 1))
                h2 = h2pool.tile([P, H], F32R, tag="h2")
                nc.scalar.activation(h2[:], o2[:],
                                     mybir.ActivationFunctionType.Relu)
                # per-sample masked pooling: msk tile is [token, 16] 0/1
                nc.tensor.matmul(ppooled[:], msk[:, gi * B:(gi + 1) * B], h2[:],
                                 start=(gi == 0), stop=(gi == t_tiles - 1),
                                 skip_group_check=True)

        def _body(_iv=None):
            ppooled = ppool.tile([B, H], F32, tag="pool")
            for (t0, n_t) in chunk_plan:
                hT = htpool.tile([P, FC * CWMAX], F32R, tag="hT")
                for s in range(n_t):
                    ti = t0 + s
                    tw = last_tw if ti == t_tiles - 1 else P
                    # the partial final tile runs an all-PE mix: PSUM rows
                    # >= tw come out zero, so no stale SBUF is ever read
                    n_pe = n_layers if tw < P else n_pe_layers
                    xt = xpool.tile([P, n_layers * F], XDT, tag="xt")
                    deng = nc.sync if (dma_engines == 1 or ti % 2 == 0) else nc.scalar
                    tb = bounds
                    if last_tile_cut and ti == t_tiles - 1:
                        tb = sorted(set(bounds) | {max(0, n_layers - last_tile_cut)})
                    for lo, hi in zip(tb[:-1], tb[1:]):
                        deng.dma_start(
                            xt[0:tw, lo * F:hi * F],
                            xp_d[ti * P: ti * P + tw, lo * F:hi * F])
                    pm0 = pmix0.tile([P, 512], F32, tag="pm0")
                    pm1 = pmix1.tile([P, F - 512], F32, tag="pm1")
                    accd = None
                    for l in range(n_layers):
                        if l < n_pe:
                            se = seye[0:tw, l * P:(l + 1) * P]
                            st, sp = (l == 0), (l == n_pe - 1)
                            nc.tensor.matmul(pm0[:], se, xt[0:tw, l * F: l * F + 512],
                                             start=st, stop=sp)
                            nc.tensor.matmul(pm1[:], se, xt[0:tw, l * F + 512:(l + 1) * F],
                                             start=st, stop=sp)
                        else:
                            xf = xt[:, l * F:(l + 1) * F].bitcast(F32)
                            sc = svec[:, l:l + 1]
                            if accd is None:
                                accd = hpool.tile([P, F], F32, tag="accd")
                                nc.vector.tensor_scalar_mul(accd[:], xf, sc)
                            else:
                                nc.vector.scalar_tensor_tensor(
                                    accd[:], xf, sc, accd[:],
                                    op0=mybir.AluOpType.mult,
                                    op1=mybir.AluOpType.add)
                    # PSUM (+ DVE partial) -> SBUF mixed tile
                    h = hpool.tile([P, F], F32, tag="h")
                    if accd is None:
                        nc.scalar.copy(h[:, 0:512], pm0[:])
                        nc.scalar.copy(h[:, 512:F], pm1[:])
                    else:
                        nc.vector.scalar_tensor_tensor(
                            h[:, 0:512], pm0[:], 1.0, accd[:, 0:512],
                            op0=mybir.AluOpType.bypass, op1=mybir.AluOpType.add)
                        nc.vector.scalar_tensor_tensor(
                            h[:, 512:F], pm1[:], 1.0, accd[:, 512:F],
                            op0=mybir.AluOpType.bypass, op1=mybir.AluOpType.add)
                    # transpose 128x128 blocks into hT
                    for fc in range(FC):
                        pt = ptr.tile([P, P], F32, tag="po")
                        nc.tensor.transpose(pt[:], h[:, fc * P:(fc + 1) * P], ident[:])
                        dst = hT[:, fc * CWMAX + s * P: fc * CWMAX + (s + 1) * P]
                        if fc % 2 == 0:
                            nc.scalar.copy(dst, pt[:])
                        else:
                            nc.vector.tensor_copy(dst, pt[:])
                mlp_chunk(hT, t0, n_t, ppooled)
            nc.scalar.copy(pooled_sb[:], ppooled[:])

        if hw_loop_repeat is not None and hw_loop_repeat > 1:
            # multiple unrolled passes per For_i iteration let the Tile
            # scheduler overlap one pass's tail with the next pass's DMA
            # (the loop body drains all engines at each iteration boundary)
            with tc.For_i(0, hw_loop_repeat, 1) as _i:
                for _u in range(unroll):
                    _body(_i)
        else:
            for _u in range(unroll):
                _body()

        nc.sync.dma_start(out_d[:], pooled_sb[:])

    if split_waits:
        _split_excess_waits(nc, max_waits=1)
    return nc


class TileKernel:
    """TileContext + ExitStack in one `with`."""

    def __init__(self, nc):
        self.tc = tile.TileContext(nc)
        self.ctx = ExitStack()

    def __enter__(self):
        tc = self.tc.__enter__()
        self.ctx.__enter__()
        return tc, self.ctx

    def __exit__(self, *exc):
        self.ctx.__exit__(*exc)
        return self.tc.__exit__(*exc)


_PROGRAM_CACHE: dict[tuple, bass.Bass] = {}


def _get_program(n_layers: int, t_tiles: int, last_tw: int) -> bass.Bass:
    key = (n_layers, t_tiles, last_tw)
    if key not in _PROGRAM_CACHE:
        _PROGRAM_CACHE[key] = build_program(n_layers, t_tiles, last_tw)
    return _PROGRAM_CACHE[key]


def _softmax32(v: np.ndarray) -> np.ndarray:
    v = v.astype(np.float32)
    e = np.exp(v - v.max())
    return (e / e.sum()).astype(np.float32)


def _prep_in_maps(inputs: dict, n_layers: int):
    x = np.asarray(inputs["x"])
    lengths = np.asarray(inputs["lengths"]).astype(np.int64)

    t_tiles, last_tw, bs, ts, val = _plan_packing(lengths)
    cap = (t_tiles - 1) * P + last_tw

    # host-side prep of the small replicated operands
    s = (_softmax32(np.asarray(inputs["mixing_weights"]))
         * np.float32(np.asarray(inputs["gamma"]).reshape(-1)[0]))
    seye = np.zeros((P, n_layers * P), np.float32)
    for l in range(n_layers):
        seye[:, l * P:(l + 1) * P] = np.eye(P, dtype=np.float32) * s[l]
    if X_BF16:
        import ml_dtypes
        seye = seye.astype(ml_dtypes.bfloat16)
    svec = np.tile(s[:n_layers], (P, 1)).astype(np.float32)
    ident = np.eye(P, dtype=np.float32)

    W1 = np.asarray(inputs["W1"], np.float32)  # [H, F]
    W2 = np.asarray(inputs["W2"], np.float32)  # [H, H]
    w1t = np.ascontiguousarray(
        W1.T.reshape(FC, P, H).transpose(1, 0, 2).reshape(P, FC * H))
    w2t = np.ascontiguousarray(
        W2.T.reshape(HC, P, H).transpose(1, 0, 2).reshape(P, HC * H))
    b1p = np.ascontiguousarray(np.asarray(inputs["b1"], np.float32).reshape(HC, P).T)
    b2rep = np.tile(np.asarray(inputs["b2"], np.float32).reshape(1, H) / P, (P, 1))
    onesm = np.ones((P, P), np.float32)

    in_maps = []
    for c in range(N_CORES):
        xp = np.ascontiguousarray(
            x[bs[c], :n_layers, ts[c], :].reshape(cap, n_layers * F))
        if X_BF16:
            import ml_dtypes
            xp = xp.astype(ml_dtypes.bfloat16)
        mm = np.zeros((t_tiles * P, B), np.float32)
        mm[np.arange(cap), bs[c]] = val[c]
        mskp = np.ascontiguousarray(
            mm.reshape(t_tiles, P, B).transpose(1, 0, 2).reshape(P, t_tiles * B))
        in_maps.append({
            "xp": xp, "seye": seye, "svec": svec, "ident": ident,
            "w1t": w1t, "w2t": w2t, "b1": b1p, "b2rep": b2rep,
            "ones": onesm, "msk": mskp,
        })
    return in_maps, dict(t_tiles=t_tiles, last_tw=last_tw)


def _finish(pooled_parts, inputs):
    lengths = np.asarray(inputs["lengths"]).astype(np.float32)
    Wl = np.asarray(inputs["Wl"], np.float32)
    bl = np.asarray(inputs["bl"], np.float32)
    pooled = np.sum(np.stack(pooled_parts, 0), axis=0, dtype=np.float32)
    pooled = pooled / lengths[:, None]
    return (pooled @ Wl.T + bl).astype(np.float32)


def kernel(x, lengths, layer, gamma, mixing_weights, W1, b1, W2, b2, Wl, bl):
    n_layers = int(np.asarray(layer)) + 1
    assert 1 <= n_layers <= L

    inputs = dict(x=x, lengths=lengths, gamma=gamma,
                  mixing_weights=mixing_weights,
                  W1=W1, b1=b1, W2=W2, b2=b2, Wl=Wl, bl=bl)
    in_maps, pa = _prep_in_maps(inputs, n_layers)
    nc = _get_program(n_layers, pa["t_tiles"], pa["last_tw"])

    res = run_bass_kernel_spmd(nc, in_maps, list(range(N_CORES)))
    return _finish([res.results[c]["out"] for c in range(N_CORES)], inputs)
